# revision 1
# baseline (speedup 1.0000x reference)
"""Bass/Trainium2 kernel for nn_GPREDecoder (GlobalPointer relation-extraction loss).

Strategy: data-parallel over batch (B=8 -> 8 cores, 1 example per core).
The device computes, per example:
  - projT = W_all @ x_aug.T  (channel-major projection, bias folded in)
  - RoPE rotation for the two "ent" heads (J-matmul + cos/sin elementwise)
  - per-head S x S logits tiles on PE, exp(SCALE*logit) on ACT with fused
    per-row accumulation -> per-head sum(exp(masked logits))  (never
    materializing the S x S tensors in HBM)
  - outputs the per-head exp-sums and the final q/k tensors
The host gathers the 64 ground-truth pairs per head from q/k, applies the
multilabel-CE pos/neg log corrections in float64, and returns the scalar loss.
"""

import ml_dtypes
import numpy as np
from contextlib import ExitStack

import concourse.bass as bass
import concourse.mybir as mybir
import concourse.tile as tile
from concourse import bacc
from concourse.bass_utils import run_bass_kernel_spmd

B, S, HID, LAB = 8, 1024, 1024, 64
HD = 68
SCALE = 1.0 / HD**0.5
INF = 1.0e12
NCORES = 8
KPAD = 1152  # 9 * 128 contraction rows (1088 channels + 1 bias row + pad)
MTOT = 544   # total projection output channels
NEG_BIG = -1.0e9  # additive pre-scale mask; exp(SCALE*NEG_BIG) == 0 in fp32

# group order: q_ent0 k_ent0 q_ent1 k_ent1 q_head k_head q_tail k_tail
_GROUP_ORIG = [0, 68, 136, 204, 272, 340, 408, 476]
# heads: (q_group, k_group, tril?)
_HEADS = [(0, 1, True), (2, 3, True), (4, 5, False), (6, 7, False)]


def _spill_slots():
    """Destination (tile, row) slots for the 4 spill groups, in order."""
    slots = []
    for t in range(4):
        slots.extend((t, r) for r in range(68, 128))
    slots.extend((4, r) for r in range(32))
    return slots


def _build_perm():
    """perm[c_new] = original channel index, for the projection output layout."""
    perm = np.zeros(MTOT, np.int64)
    for g in range(4):  # rope groups aligned at row 0 of tiles 0..3
        perm[g * 128: g * 128 + 68] = np.arange(_GROUP_ORIG[g], _GROUP_ORIG[g] + 68)
    slots = _spill_slots()
    pos = 0
    for g in range(4, 8):
        for j in range(68):
            t, r = slots[pos]
            perm[t * 128 + r] = _GROUP_ORIG[g] + j
            pos += 1
    return perm


def _spill_pieces():
    """Per spill group: contiguous (src_tile, src_row0, cnt, dst_row0) DMA pieces."""
    slots = _spill_slots()
    out = {g: [] for g in range(4, 8)}
    pos = 0
    for g in range(4, 8):
        j = 0
        while j < 68:
            t, r = slots[pos]
            cnt = 1
            while j + cnt < 68 and pos + cnt < len(slots) and \
                    slots[pos + cnt] == (t, r + cnt):
                cnt += 1
            out[g].append((t, r, cnt, j))
            pos += cnt
            j += cnt
    return out


def _round_chunks(mtiles):
    """Chunk m-tiles of one [128,1024] psum round into bank-fitting matmul chunks.

    mtiles: [(m, local_start, width)] with local starts such that every
    <=512 chunk stays inside one 512-col bank. Returns
    [(m, local_off, src_off, n)] and the single contiguous ACT span end.
    """
    chunks = []
    for (m, lo, w) in mtiles:
        off = 0
        while off < w:
            n = min(512 - ((lo + off) % 512), w - off)
            chunks.append((m, lo + off, off, n))
            off += n
    return chunks


def _head_rounds(is_tril):
    """Per head: list of rounds; each round = (mtiles, span_end).

    Rounds target [128, 1024] (2-bank) psum tiles. For tril heads the
    m-tile widths shrink (only columns >= 128*m are live), so later
    m-tiles are packed two per round; spans stay contiguous from 0.
    """
    if not is_tril:
        return [([(m, 0, 1024)], 1024) for m in range(8)]
    widths = [1024 - 128 * m for m in range(8)]
    rounds = []
    for group in ((0,), (1,), (2, 6), (3, 7), (4, 5)):
        mtiles = []
        local = 0
        for m in group:
            mtiles.append((m, local, widths[m]))
            local += widths[m]
        rounds.append((mtiles, local))
    return rounds


def _n_act_cols(is_tril):
    return len(_head_rounds(is_tril))


_ACC_COLS = [_n_act_cols(t) for _, _, t in _HEADS]          # per head
_ACC_OFF = np.concatenate([[0], np.cumsum(_ACC_COLS)])      # col offset per head
SUMS_COLS = int(_ACC_OFF[-1])                               # total accum columns


def _build_nc():
    f32 = mybir.dt.float32
    # float32r: same 4-byte storage, but the PE streams it at full rate
    # (strict fp32 runs as 2 half-speed passes = 4x slower). The q/k logits
    # path jmat -> dense -> qk is typed f32r end-to-end. The projection
    # inputs are bf16 to halve the HBM load volume.
    f32r = mybir.dt.float32r
    bf16 = mybir.dt.bfloat16
    Exp = mybir.ActivationFunctionType.Exp

    nc = bacc.Bacc("TRN2", target_bir_lowering=False)

    xT = nc.dram_tensor("xT", [KPAD, S], bf16, kind="ExternalInput")
    wtb = nc.dram_tensor("wtb", [KPAD, MTOT], bf16, kind="ExternalInput")
    trig = nc.dram_tensor("trig", [HD, 2 * S], f32, kind="ExternalInput")
    jtril = nc.dram_tensor("jtril", [128, 256], f32r, kind="ExternalInput")
    sums = nc.dram_tensor("sums", [128, SUMS_COLS], f32, kind="ExternalOutput")
    qkout = nc.dram_tensor("qkout", [8, HD, S], f32r, kind="ExternalOutput")

    xT_r = xT.rearrange("(o p) f -> p o f", p=128)    # [128, 9, 1024]
    wtb_r = wtb.rearrange("(o p) f -> p o f", p=128)  # [128, 9, 544]
    KT_CHUNKS = [(0, 2), (2, 3), (5, 4)]              # (kt0, n_kt) DMA chunks

    with tile.TileContext(nc) as tc, ExitStack() as ctx:
        singles = ctx.enter_context(tc.tile_pool(name="singles", bufs=1))
        scratch = ctx.enter_context(tc.tile_pool(name="scratch", bufs=2))

        xT_sb = singles.tile([128, 9, S], bf16, tag="xT_sb", name="xT_sb")
        wtb_sb = singles.tile([128, 9, MTOT], bf16, tag="wtb_sb", name="wtb_sb")
        trig_sb = singles.tile([HD, 2 * S], f32, tag="trig_sb", name="trig_sb")
        jtril_sb = singles.tile([128, 256], f32r, tag="jtril_sb", name="jtril_sb")
        dense = [singles.tile([128, S], f32r, tag=f"dense{t}", name=f"dense{t}")
                 for t in range(5)]
        qk = [singles.tile([HD, S], f32r, tag=f"qk{g}", name=f"qk{g}")
              for g in range(8)]
        sums_sb = singles.tile([128, SUMS_COLS], f32, tag="sums_sb", name="sums_sb")
        dummy = singles.tile([1, 8], f32, tag="dummy", name="dummy")

        cos_sb = trig_sb[:, 0:S]
        sin_sb = trig_sb[:, S:2 * S]
        jmat_sb = jtril_sb[:, 0:128]
        tril_sb = jtril_sb[:, 128:256].bitcast(f32)

        # Early: zero accumulators; pre-warm the ACT exp table load.
        nc.vector.memset(sums_sb[:], 0.0)
        nc.vector.memset(dummy[:], 0.0)
        nc.scalar.activation(dummy[:], dummy[:], Exp)

        # input DMAs: first kt chunk first so the projection starts ASAP;
        # constants (needed only ~10us in) after the first chunk.
        def in_chunk(ci):
            kt0, nkt = KT_CHUNKS[ci]
            nc.sync.dma_start(out=wtb_sb[:, kt0:kt0 + nkt],
                              in_=wtb_r[:, kt0:kt0 + nkt])
            nc.scalar.dma_start(out=xT_sb[:, kt0:kt0 + nkt],
                                in_=xT_r[:, kt0:kt0 + nkt])

        in_chunk(0)
        nc.sync.dma_start(out=jtril_sb[:], in_=jtril[:, :])
        nc.scalar.dma_start(out=trig_sb[:], in_=trig[:, :])
        in_chunk(1)
        in_chunk(2)

        ps = ctx.enter_context(tc.tile_pool(name="ps", bufs=4, space="PSUM"))

        def proj_tile(t, pt, kt_lo=0, kt_hi=9):
            lo = t * 128
            hi = min(lo + 128, MTOT)
            for kt in range(kt_lo, kt_hi):
                for c in (0, 512):
                    nc.tensor.matmul(
                        pt[0:hi - lo, c:c + 512],
                        wtb_sb[:, kt, lo:hi],
                        xT_sb[:, kt, c:c + 512],
                        start=(kt == 0), stop=(kt == 8),
                    )

        def evac(t, pt, eng):
            hi = min(128, MTOT - t * 128)
            if eng == "act":
                nc.scalar.copy(out=dense[t][0:hi, :], in_=pt[0:hi, :])
            else:
                nc.vector.tensor_copy(out=dense[t][0:hi, :], in_=pt[0:hi, :])

        def jrot(g):
            """J-matmul for rope group g; returns the psum tile to release."""
            pj = ps.tile([128, S], f32, tag="ps", name=f"jq{g}")
            for c in (0, 512):
                nc.tensor.matmul(pj[:, c:c + 512], jmat_sb,
                                 dense[g][:, c:c + 512], start=True, stop=True)
            return pj

        def rope(g, pj):
            # qk[g] = dense[g]*cos + (J @ dense[g])*sin
            nc.gpsimd.tensor_tensor(qk[g][:, :], dense[g][0:HD, :], cos_sb,
                                    mybir.AluOpType.mult)
            rtmp = scratch.tile([HD, S], f32, tag="rtmp", name=f"rtmp{g}")
            nc.vector.tensor_tensor(rtmp[:, :], pj[0:HD, :], sin_sb,
                                    mybir.AluOpType.mult)
            nc.vector.tensor_tensor(qk[g][:, :], qk[g][:, :], rtmp[:, :],
                                    mybir.AluOpType.add)

        def head_logits(h, interleave=None):
            gq, gk, is_tril = _HEADS[h]
            acc = int(_ACC_OFF[h])
            for ri, (mtiles, span_end) in enumerate(_head_rounds(is_tril)):
                pl = ps.tile([128, S], f32, tag="ps", name=f"l{h}_{ri}")
                for (m, lo, so, n) in _round_chunks(mtiles):
                    g0 = 128 * m if is_tril else 0
                    nc.tensor.matmul(
                        pl[:, lo:lo + n],
                        qk[gq][:, m * 128:(m + 1) * 128],
                        qk[gk][:, g0 + so:g0 + so + n],
                        start=True, stop=True,
                    )
                if is_tril:
                    for (m, lo, w) in mtiles:
                        nc.vector.tensor_tensor(
                            pl[:, lo:lo + 128], pl[:, lo:lo + 128],
                            tril_sb, mybir.AluOpType.add)
                nc.scalar.activation(
                    pl[:, 0:span_end], pl[:, 0:span_end], Exp, scale=SCALE,
                    accum_out=sums_sb[:, acc:acc + 1])
                acc += 1
                if interleave is not None:
                    interleave(ri)
            assert acc == int(_ACC_OFF[h + 1])

        # ---- phase B1: projection tiles 0,1 (the ent-h0 rope groups) ----
        pt0 = ps.tile([128, S], f32, tag="ps", name="proj0")
        pt1 = ps.tile([128, S], f32, tag="ps", name="proj1")
        for kt in range(9):
            for t, pt in ((0, pt0), (1, pt1)):
                for c in (0, 512):
                    nc.tensor.matmul(pt[:, c:c + 512],
                                     wtb_sb[:, kt, t * 128:(t + 1) * 128],
                                     xT_sb[:, kt, c:c + 512],
                                     start=(kt == 0), stop=(kt == 8))
        evac(0, pt0, "dve")
        evac(1, pt1, "dve")
        pj0 = jrot(0)
        pj1 = jrot(1)
        rope(0, pj0)
        rope(1, pj1)
        nc.sync.dma_start(out=qkout[0], in_=qk[0][:, :])
        nc.scalar.dma_start(out=qkout[1], in_=qk[1][:, :])

        # ---- ent head 0: starts the ACT exp stream as early as possible ----
        head_logits(0)

        # ---- phase B2: projection tiles 2,3 ----
        pt2 = ps.tile([128, S], f32, tag="ps", name="proj2")
        pt3 = ps.tile([128, S], f32, tag="ps", name="proj3")
        proj_tile(2, pt2)
        proj_tile(3, pt3)
        evac(2, pt2, "dve")
        evac(3, pt3, "dve")

        # ---- phase B3: projection tile 4 + spill regroup for head/tail ----
        pt4 = ps.tile([128, S], f32, tag="ps", name="proj4")
        proj_tile(4, pt4)
        evac(4, pt4, "dve")
        for g, pieces in _spill_pieces().items():
            for i, (t, r0, cnt, d0) in enumerate(pieces):
                eng = nc.sync if (g + i) % 2 == 0 else nc.scalar
                eng.dma_start(out=qk[g][d0:d0 + cnt, :],
                              in_=dense[t][r0:r0 + cnt, :])
            eng = nc.sync if g % 2 == 0 else nc.scalar
            eng.dma_start(out=qkout[g], in_=qk[g][:, :])

        # ---- rope for ent head 1 while the head/tail spill DMAs run ----
        pj2 = jrot(2)
        pj3 = jrot(3)
        rope(2, pj2)
        rope(3, pj3)
        nc.sync.dma_start(out=qkout[2], in_=qk[2][:, :])
        nc.scalar.dma_start(out=qkout[3], in_=qk[3][:, :])

        # ---- remaining heads: head first (its deps finish earliest) ----
        head_logits(2)
        head_logits(1)
        head_logits(3)

        nc.sync.dma_start(out=sums[:, :], in_=sums_sb[:, :])

    nc.finalize()
    return nc


_NC_CACHE = None


def _get_nc():
    global _NC_CACHE
    if _NC_CACHE is None:
        _NC_CACHE = _build_nc()
    return _NC_CACHE


def _host_tables():
    pos = np.arange(S, dtype=np.float64)[:, None]
    inv = np.power(10000.0, -2.0 * np.arange(HD // 2, dtype=np.float64) / HD)
    ang = pos * inv                                   # [S, 34]
    trig = np.zeros((HD, 2 * S), np.float32)
    trig[:, 0:S] = np.repeat(np.cos(ang), 2, axis=1).T
    trig[:, S:2 * S] = np.repeat(np.sin(ang), 2, axis=1).T
    jtril = np.zeros((128, 256), np.float32)          # [:, :128]=J.T, [:, 128:]=tril
    for i in range(HD // 2):
        # J[2i, 2i+1] = -1 ; J[2i+1, 2i] = +1  -> stored transposed
        jtril[2 * i + 1, 2 * i] = -1.0
        jtril[2 * i, 2 * i + 1] = 1.0
    jtril[:, 128:256] = np.where(
        np.arange(128)[None, :] >= np.arange(128)[:, None], 0.0, NEG_BIG)
    return trig, jtril


def _mcce_host(E_dev, q, k, gt):
    """pos/neg multilabel-CE for one (example, head). q,k: [68,S] f32; gt: [P,2]."""
    i = gt[:, 0].astype(np.int64)
    j = gt[:, 1].astype(np.int64)
    flat = i * S + j
    lv = np.sum(q[:, i].astype(np.float64) * k[:, j].astype(np.float64),
                axis=0) * SCALE                       # [P]
    live = flat != 0
    pos_loss = np.log1p(np.sum(np.exp(-lv[live])))
    l00 = float(np.sum(q[:, 0].astype(np.float64) * k[:, 0].astype(np.float64))
                * SCALE)
    uf, ui = np.unique(flat, return_index=True)
    keep = uf != 0
    excl = np.exp(l00) + np.sum(np.exp(lv[ui[keep]]))
    neg_loss = np.log1p(E_dev - excl)
    return pos_loss + neg_loss


def _reference_numpy(hidden, entity_labels, attention_mask, gt_entity, gt_head,
                     gt_tail, ent_emb, W_ent, b_ent, W_head, b_head, W_tail,
                     b_tail):
    """Slow exact numpy fallback (used only if attention_mask is not all-ones)."""
    x = np.concatenate([hidden, ent_emb[entity_labels]], axis=-1)

    def rope(v):
        b, s, h, d = v.shape
        pos = np.arange(s, dtype=np.float32)[:, None]
        inv = np.power(10000.0, -2.0 * np.arange(d // 2, dtype=np.float32) / d)
        ang = pos * inv
        sin = np.repeat(np.sin(ang), 2, axis=-1)[None, :, None, :]
        cos = np.repeat(np.cos(ang), 2, axis=-1)[None, :, None, :]
        v2 = np.stack([-v[..., 1::2], v[..., ::2]], axis=-1).reshape(v.shape)
        return v * cos + v2 * sin

    def gp(x, W, b, mask, heads, use_rope, tril):
        bx, sx, _ = x.shape
        proj = (x @ W.T + b).reshape(bx, sx, heads, 2 * HD)
        qw, kw = proj[..., :HD], proj[..., HD:]
        if use_rope:
            qw, kw = rope(qw), rope(kw)
        logits = np.einsum('bmhd,bnhd->bhmn', qw, kw) * SCALE
        pad = mask[:, None, None, :]
        logits = logits * pad - (1.0 - pad) * INF
        if tril:
            logits = logits - np.tril(np.ones((sx, sx), np.float32), -1) * INF
        return logits

    def mcce(y_true, y_pred):
        bx, hx, sx, _ = y_pred.shape
        flat = y_true[..., 0].astype(np.int64) * sx + y_true[..., 1]
        yp = y_pred.reshape(bx, hx, sx * sx).astype(np.float64)
        total = 0.0
        for b in range(bx):
            for h in range(hx):
                f = flat[b, h]
                live = f != 0
                lv = yp[b, h][f]
                pos = np.log1p(np.sum(np.exp(-lv[live])))
                neg_terms = yp[b, h].copy()
                neg_terms[0] = -np.inf
                neg_terms[np.unique(f)] = -np.inf
                neg = np.log1p(np.sum(np.exp(neg_terms)))
                total += pos + neg
        return total

    loss = 0.0
    loss += mcce(gt_entity, gp(x, W_ent, b_ent, attention_mask, 2, True, True))
    loss += mcce(gt_head, gp(x, W_head, b_head, attention_mask, 1, False, False))
    loss += mcce(gt_tail, gp(x, W_tail, b_tail, attention_mask, 1, False, False))
    return np.array(loss, dtype=np.float32)


def kernel(hidden, entity_labels, attention_mask, gt_entity, gt_head, gt_tail,
           ent_emb, W_ent, b_ent, W_head, b_head, W_tail, b_tail,
           _want_trace=False):
    hidden = np.asarray(hidden, np.float32)
    entity_labels = np.asarray(entity_labels)
    attention_mask = np.asarray(attention_mask, np.float32)
    ent_emb = np.asarray(ent_emb, np.float32)

    if not np.all(attention_mask == 1.0):
        return _reference_numpy(
            hidden, entity_labels, attention_mask, np.asarray(gt_entity),
            np.asarray(gt_head), np.asarray(gt_tail), ent_emb,
            np.asarray(W_ent, np.float32), np.asarray(b_ent, np.float32),
            np.asarray(W_head, np.float32), np.asarray(b_head, np.float32),
            np.asarray(W_tail, np.float32), np.asarray(b_tail, np.float32))

    W_all = np.concatenate(
        [np.asarray(W_ent, np.float32), np.asarray(W_head, np.float32),
         np.asarray(W_tail, np.float32)], axis=0)       # [544, 1088]
    b_all = np.concatenate(
        [np.asarray(b_ent, np.float32), np.asarray(b_head, np.float32),
         np.asarray(b_tail, np.float32)], axis=0)       # [544]
    perm = _build_perm()
    Wp, bp = W_all[perm], b_all[perm]
    wtb = np.zeros((KPAD, MTOT), np.float32)
    wtb[:HID + LAB] = Wp.T
    wtb[HID + LAB] = bp
    wtb = wtb.astype(ml_dtypes.bfloat16)

    trig, jtril = _host_tables()

    in_maps = []
    for b in range(B):
        xT = np.zeros((KPAD, S), np.float32)
        xT[:HID] = hidden[b].T
        xT[HID:HID + LAB] = ent_emb[entity_labels[b]].T
        xT[HID + LAB] = 1.0
        in_maps.append(dict(xT=xT.astype(ml_dtypes.bfloat16), wtb=wtb,
                            trig=trig, jtril=jtril))

    nc = _get_nc()
    res = run_bass_kernel_spmd(nc, in_maps, core_ids=list(range(NCORES)),
                               trace=_want_trace)

    gts = {0: np.asarray(gt_entity), 2: np.asarray(gt_head),
           3: np.asarray(gt_tail)}
    total = 0.0
    for b in range(B):
        out = res.results[b]
        sums = out["sums"].astype(np.float64)      # [128, SUMS_COLS]
        qkv = out["qkout"]                         # [8, 68, 1024]
        for h, (gq, gk, is_tril) in enumerate(_HEADS):
            E = float(np.sum(sums[:, _ACC_OFF[h]:_ACC_OFF[h + 1]]))
            if h < 2:
                gt = gts[0][b, h]
            else:
                gt = gts[h][b, 0]
            total += _mcce_host(E, qkv[gq], qkv[gk], gt)

    if _want_trace:
        kernel._last_results = res
    return np.array(total, dtype=np.float32)



# revision 3
# speedup vs baseline: 1.4662x; 1.4662x over previous
"""Bass/Trainium2 kernel for nn_GPREDecoder (GlobalPointer relation-extraction loss).

Strategy: data-parallel over batch (B=8 -> 8 cores, 1 example per core).
Per example the device computes:
  - fp8 DoubleRow projection  projT = (alpha*W_all) @ x_aug.T  (bias folded),
    output channels permuted so q_head/k_head/q_ent0/k_ent0/q_ent1 land at
    row 0 of the five 128-row psum m-tiles (direct SBUF views after a bf16
    evacuation cast); the 3 remaining groups are regrouped by SBUF->SBUF DMA.
  - RoPE on DVE: stream_shuffle pair-swap + sign-folded sin table
    (rot(q) = q*cos + swap(q)*sin'), no J matmul, all bf16 2x-mode ops.
  - per-head S x S logits in bf16 on PE, tril masks added by identity-matmul
    accumulation, exp(SCALE/alpha^2 * logit) on ACT with fused per-row
    accumulation into a [128, 14] sums tile (2048-wide spans).
  - bf16 q/k tensors DMA'd out for the host-side multilabel-CE corrections
    (gathers of the 64 ground-truth pairs, computed in float64).
"""

import ml_dtypes
import numpy as np
from contextlib import ExitStack

import concourse.bass as bass
import concourse.mybir as mybir
import concourse.tile as tile
from concourse import bacc
from concourse.bass_utils import run_bass_kernel_spmd

B, S, HID, LAB = 8, 1024, 1024, 64
HD = 68
SCALE = 1.0 / HD**0.5
INF = 1.0e12
NCORES = 8
ALPHA = 16.0                  # fp8 weight pre-scale; exp scale divides alpha^2
ACT_SCALE = SCALE / (ALPHA * ALPHA)
NEG_BIG = -1.0e9 * ALPHA * ALPHA  # additive mask units match scaled logits
KPAD = 1152                   # 4 full double-row k-tiles (256ch) + 1 half (128ch)
MSLOT = 640                   # 5 m-tiles x 128 permuted output-channel slots

# qkout group order (our choice):
G_QHEAD, G_KHEAD, G_QTAIL, G_KTAIL, G_QE0, G_KE0, G_QE1, G_KE1 = range(8)
# original row offset of each 68-row group in W_all = [W_ent; W_head; W_tail]
_ORIG = {G_QE0: 0, G_KE0: 68, G_QE1: 136, G_KE1: 204,
         G_QHEAD: 272, G_KHEAD: 340, G_QTAIL: 408, G_KTAIL: 476}
# row-0 groups of m-tiles 0..4 (direct views of the dense tiles)
_ROW0 = [G_QHEAD, G_KHEAD, G_QE0, G_KE0, G_QE1]
# spill groups: (src_tile, src_row, cnt, dst_row) pieces
_SPILL = {
    G_KE1: [(0, 68, 60, 0), (1, 68, 8, 60)],
    G_QTAIL: [(1, 76, 52, 0), (2, 68, 16, 52)],
    G_KTAIL: [(2, 84, 44, 0), (3, 68, 24, 44)],
}
# tril m-tile widths and ACT packs: lists of (m, local_col) per pack
_TRIL_W = [S - 128 * m for m in range(8)]
_TRIL_PACKS = [
    [(0, 0), (1, 1024)],            # span 1920
    [(2, 0), (3, 768), (4, 1408)],  # span 1920
    [(5, 0), (6, 384), (7, 640)],   # span 768
]
_TRIL_SPANS = [1920, 1920, 768]

# heads: (name, q_operand, k_operand, tril?) resolved at build time
# ACT instruction order: head x4 pairs, tail x4 pairs, ent0 x3, ent1 x3
N_ACC = 14


def _slot_map():
    """slot (0..639) -> original W_all row, or -1 for pad."""
    slot = np.full(MSLOT, -1, np.int64)
    for t, g in enumerate(_ROW0):
        slot[t * 128: t * 128 + 68] = np.arange(_ORIG[g], _ORIG[g] + 68)
    for g, pieces in _SPILL.items():
        for (t, r, cnt, d) in pieces:
            slot[t * 128 + r: t * 128 + r + cnt] = np.arange(
                _ORIG[g] + d, _ORIG[g] + d + cnt)
    return slot


def _chunks_in_bank(base, lo, w):
    """Split [lo, lo+w) cols (absolute base+lo in the psum tile) at the
    512-col bank grid. Returns [(off, n)] with off relative to lo."""
    out = []
    off = 0
    while off < w:
        a = base + lo + off
        n = min(512 - (a % 512), w - off)
        out.append((off, n))
        off += n
    return out


def _build_nc():
    f32 = mybir.dt.float32
    bf16 = mybir.dt.bfloat16
    f8 = mybir.dt.float8e4
    Exp = mybir.ActivationFunctionType.Exp
    DR = mybir.MatmulPerfMode.DoubleRow

    nc = bacc.Bacc("TRN2", target_bir_lowering=False)

    xT8a = nc.dram_tensor("xT8a", [128, 8192], f8, kind="ExternalInput")
    xT8b = nc.dram_tensor("xT8b", [64, 2048], f8, kind="ExternalInput")
    wtb8a = nc.dram_tensor("wtb8a", [128, 5120], f8, kind="ExternalInput")
    wtb8b = nc.dram_tensor("wtb8b", [64, 1280], f8, kind="ExternalInput")
    cosT = nc.dram_tensor("cosT", [HD, S], bf16, kind="ExternalInput")
    sinT = nc.dram_tensor("sinT", [HD, S], bf16, kind="ExternalInput")
    aux = nc.dram_tensor("aux", [128, 256], bf16, kind="ExternalInput")
    sums = nc.dram_tensor("sums", [128, N_ACC], f32, kind="ExternalOutput")
    qkout = nc.dram_tensor("qkout", [8, HD, S], bf16, kind="ExternalOutput")

    with tile.TileContext(nc) as tc, ExitStack() as ctx:
        singles = ctx.enter_context(tc.tile_pool(name="singles", bufs=1))
        scratch = ctx.enter_context(tc.tile_pool(name="scratch", bufs=2))

        xT_sb = singles.tile([128, 5, 2, S], f8, tag="xT_sb", name="xT_sb")
        wtb_sb = singles.tile([128, 5, 2, MSLOT], f8, tag="wtb_sb", name="wtb_sb")
        cos_sb = singles.tile([HD, S], bf16, tag="cos_sb", name="cos_sb")
        sin_sb = singles.tile([HD, S], bf16, tag="sin_sb", name="sin_sb")
        aux_sb = singles.tile([128, 256], bf16, tag="aux_sb", name="aux_sb")
        dense = [singles.tile([128, S], bf16, tag=f"dense{t}", name=f"dense{t}")
                 for t in range(5)]
        qk_sp = {g: singles.tile([HD, S], bf16, tag=f"sp{g}", name=f"sp{g}")
                 for g in _SPILL}
        qk_rope = {g: singles.tile([HD, S], bf16, tag=f"rp{g}", name=f"rp{g}")
                   for g in (G_QE0, G_KE0, G_QE1, G_KE1)}
        sums_sb = singles.tile([128, N_ACC], f32, tag="sums_sb", name="sums_sb")
        dummy = singles.tile([1, 8], f32, tag="dummy", name="dummy")

        I_sb = aux_sb[:, 0:128]
        mask_sb = aux_sb[:, 128:256]

        ps = ctx.enter_context(tc.tile_pool(name="ps", bufs=1, space="PSUM"))
        big = ps.tile([128, 4096], f32, tag="big", name="big")

        # ---- warm-up + input DMAs (sync + scalar HW rings) ----
        nc.vector.memset(dummy[:], 0.0)
        nc.scalar.activation(dummy[:], dummy[:], Exp)

        nc.sync.dma_start(out=wtb_sb[:, 0:2], in_=wtb8a.rearrange(
            "p (o i f) -> p o i f", o=4, i=2)[:, 0:2])
        nc.sync.dma_start(out=xT_sb[:, 0:2], in_=xT8a.rearrange(
            "p (o i f) -> p o i f", o=4, i=2)[:, 0:2])
        nc.sync.dma_start(out=xT_sb[:, 2:4], in_=xT8a.rearrange(
            "p (o i f) -> p o i f", o=4, i=2)[:, 2:4])

        nc.scalar.dma_start(out=wtb_sb[:, 2:4], in_=wtb8a.rearrange(
            "p (o i f) -> p o i f", o=4, i=2)[:, 2:4])
        nc.scalar.dma_start(out=wtb_sb[0:64, 4], in_=wtb8b.rearrange(
            "p (i f) -> p i f", i=2)[:, :, :])
        nc.scalar.dma_start(out=xT_sb[0:64, 4], in_=xT8b.rearrange(
            "p (i f) -> p i f", i=2)[:, :, :])
        nc.scalar.dma_start(out=sin_sb[:, :], in_=sinT[:, :])
        nc.scalar.dma_start(out=cos_sb[:, :], in_=cosT[:, :])
        nc.scalar.dma_start(out=aux_sb[:, :], in_=aux[:, :])

        # ---- helpers ----
        def proj(t, u):
            """Project m-tile t into psum unit u (cols u*1024..)."""
            base = u * 1024
            for kt in range(5):
                p_hi = 64 if kt == 4 else 128
                for c in (0, 512):
                    nc.tensor.matmul(
                        big[0:128, base + c: base + c + 512],
                        wtb_sb[0:p_hi, kt, :, t * 128:(t + 1) * 128],
                        xT_sb[0:p_hi, kt, :, c:c + 512],
                        start=(kt == 0), stop=(kt == 4),
                        perf_mode=DR,
                    )

        def evac(t, u, eng):
            src = big[0:128, u * 1024:(u + 1) * 1024]
            if eng == "act":
                nc.scalar.copy(out=dense[t][:, :], in_=src)
            else:
                nc.vector.tensor_copy(out=dense[t][:, :], in_=src)

        def pair_round(q_ap, k_ap, r0, u0):
            """Two non-tril rounds r0, r0+1 into units u0, u0+1 (2048 span)."""
            for j in (0, 1):
                m = r0 + j
                base = (u0 + j) * 1024
                for c in (0, 512):
                    nc.tensor.matmul(
                        big[0:128, base + c: base + c + 512],
                        q_ap[:, m * 128:(m + 1) * 128],
                        k_ap[:, c:c + 512],
                        start=True, stop=True,
                    )

        def tril_pack(q_ap, k_ap, pack, u0):
            """One tril ACT pack into units u0,u0+1; mask + logits matmuls."""
            base = u0 * 1024
            for (m, lo) in pack:
                w = _TRIL_W[m]
                g0 = 128 * m
                # diag block: mask first (start), logits joins (stop)
                nc.tensor.matmul(
                    big[0:128, base + lo: base + lo + 128],
                    I_sb, mask_sb, start=True, stop=False)
                nc.tensor.matmul(
                    big[0:128, base + lo: base + lo + 128],
                    q_ap[:, g0:g0 + 128], k_ap[:, g0:g0 + 128],
                    start=False, stop=True)
                for (off, n) in _chunks_in_bank(base, lo + 128, w - 128):
                    nc.tensor.matmul(
                        big[0:128, base + lo + 128 + off:
                            base + lo + 128 + off + n],
                        q_ap[:, g0:g0 + 128],
                        k_ap[:, g0 + 128 + off: g0 + 128 + off + n],
                        start=True, stop=True)

        acc_i = [0]

        def act_span(u0, span):
            i = acc_i[0]
            acc_i[0] += 1
            ap = big[0:128, u0 * 1024: u0 * 1024 + span]
            nc.scalar.activation(ap, ap, Exp, scale=ACT_SCALE,
                                 accum_out=sums_sb[:, i:i + 1])

        def rope(g, src_ap):
            """qk_rope[g] = src*cos + pairswap(src)*sin' (DVE)."""
            sh = scratch.tile([HD, S], bf16, tag="sh", name=f"sh{g}")
            tmp = scratch.tile([HD, S], bf16, tag="tmp", name=f"tmp{g}")
            swap_mask = [i ^ 1 for i in range(32)]
            nc.vector.stream_shuffle(sh[:, :], src_ap, swap_mask)
            nc.vector.tensor_tensor(tmp[:, :], sh[:, :], sin_sb[:, :],
                                    mybir.AluOpType.mult)
            nc.vector.tensor_tensor(qk_rope[g][:, :], src_ap, cos_sb[:, :],
                                    mybir.AluOpType.mult)
            nc.vector.tensor_tensor(qk_rope[g][:, :], qk_rope[g][:, :],
                                    tmp[:, :], mybir.AluOpType.add)

        # ---- PE: projections 0-3 ----
        proj(0, 0)
        proj(1, 1)
        proj(2, 2)
        proj(3, 3)

        # ---- evacuations: dense0 on DVE, dense1 on ACT, 2/3 on DVE ----
        evac(0, 0, "dve")
        evac(1, 1, "act")
        evac(2, 2, "dve")
        evac(3, 3, "dve")

        q_head = dense[0][0:HD, :]
        k_head = dense[1][0:HD, :]
        q_ent0_raw = dense[2][0:HD, :]
        k_ent0_raw = dense[3][0:HD, :]

        # ---- GPSIMD: spills + early qkout ----
        for g, pieces in _SPILL.items():
            for (t, r, cnt, d) in pieces:
                nc.gpsimd.dma_start(out=qk_sp[g][d:d + cnt, :],
                                    in_=dense[t][r:r + cnt, :])
        nc.gpsimd.dma_start(out=qkout[G_QHEAD], in_=q_head)
        nc.gpsimd.dma_start(out=qkout[G_KHEAD], in_=k_head)
        for g in (G_QTAIL, G_KTAIL):
            nc.gpsimd.dma_start(out=qkout[g], in_=qk_sp[g][:, :])

        # ---- DVE: rope for ent0 (ent1 after evac4) ----
        rope(G_QE0, q_ent0_raw)
        rope(G_KE0, k_ent0_raw)

        # ---- PE + ACT: head pairs A-D, proj4 squeezed into the U01 cycle ---
        pair_round(q_head, k_head, 0, 0)      # headA @U01
        act_span(0, 2048)
        pair_round(q_head, k_head, 2, 2)      # headB @U23
        act_span(2, 2048)
        pair_round(q_head, k_head, 4, 0)      # headC @U01
        act_span(0, 2048)
        pair_round(q_head, k_head, 6, 2)      # headD @U23
        act_span(2, 2048)

        proj(4, 0)                            # proj4 @U0 after headC drained
        evac(4, 0, "dve")
        rope(G_QE1, dense[4][0:HD, :])
        rope(G_KE1, qk_sp[G_KE1][:, :])
        for g in (G_QE0, G_KE0, G_QE1, G_KE1):
            nc.gpsimd.dma_start(out=qkout[g], in_=qk_rope[g][:, :])

        q_tail = qk_sp[G_QTAIL][:, :]
        k_tail = qk_sp[G_KTAIL][:, :]
        pair_round(q_tail, k_tail, 0, 2)      # tailA @U23
        act_span(2, 2048)
        pair_round(q_tail, k_tail, 2, 0)      # tailB @U01 (after evac4)
        act_span(0, 2048)
        pair_round(q_tail, k_tail, 4, 2)      # tailC @U23
        act_span(2, 2048)
        pair_round(q_tail, k_tail, 6, 0)      # tailD @U01
        act_span(0, 2048)

        qe0, ke0 = qk_rope[G_QE0][:, :], qk_rope[G_KE0][:, :]
        qe1, ke1 = qk_rope[G_QE1][:, :], qk_rope[G_KE1][:, :]
        units = [2, 0, 2, 0, 2, 0]
        plan = [(qe0, ke0, 0), (qe0, ke0, 1), (qe0, ke0, 2),
                (qe1, ke1, 0), (qe1, ke1, 1), (qe1, ke1, 2)]
        for (qa, ka, pi), u0 in zip(plan, units):
            tril_pack(qa, ka, _TRIL_PACKS[pi], u0)
            act_span(u0, _TRIL_SPANS[pi])

        nc.sync.dma_start(out=sums[:, :], in_=sums_sb[:, :])

    nc.finalize()
    return nc


_NC_CACHE = None


def _get_nc():
    global _NC_CACHE
    if _NC_CACHE is None:
        _NC_CACHE = _build_nc()
    return _NC_CACHE


def _host_tables():
    pos = np.arange(S, dtype=np.float64)[:, None]
    inv = np.power(10000.0, -2.0 * np.arange(HD // 2, dtype=np.float64) / HD)
    ang = pos * inv                                    # [S, 34]
    cosr = np.repeat(np.cos(ang), 2, axis=1).T         # [68, S]
    sinr = np.repeat(np.sin(ang), 2, axis=1).T
    # sign-folded sin: rot = x*cos + swap(x)*sin'; sin'[2i] = -sin, [2i+1] = +
    sgn = np.where(np.arange(HD) % 2 == 0, -1.0, 1.0)[:, None]
    cosT = cosr.astype(ml_dtypes.bfloat16)
    sinT = (sinr * sgn).astype(ml_dtypes.bfloat16)
    auxm = np.zeros((128, 256), np.float32)
    auxm[:, 0:128] = np.eye(128, dtype=np.float32)
    auxm[:, 128:256] = np.where(
        np.arange(128)[None, :] >= np.arange(128)[:, None], 0.0, NEG_BIG)
    return cosT, sinT, auxm.astype(ml_dtypes.bfloat16)


def _mcce_host(E_dev, q, k, gt):
    """pos/neg multilabel-CE for one (example, head). q,k: [68,S] f64; gt [P,2]."""
    i = gt[:, 0].astype(np.int64)
    j = gt[:, 1].astype(np.int64)
    flat = i * S + j
    lv = np.sum(q[:, i] * k[:, j], axis=0) * SCALE     # [P]
    live = flat != 0
    pos_loss = np.log1p(np.sum(np.exp(-lv[live])))
    l00 = float(np.sum(q[:, 0] * k[:, 0]) * SCALE)
    uf, ui = np.unique(flat, return_index=True)
    keep = uf != 0
    excl = np.exp(l00) + np.sum(np.exp(lv[ui[keep]]))
    neg_loss = np.log1p(E_dev - excl)
    return pos_loss + neg_loss


def _reference_numpy(hidden, entity_labels, attention_mask, gt_entity, gt_head,
                     gt_tail, ent_emb, W_ent, b_ent, W_head, b_head, W_tail,
                     b_tail):
    """Slow exact numpy fallback (used only if attention_mask is not all-ones)."""
    x = np.concatenate([hidden, ent_emb[entity_labels]], axis=-1)

    def rope_np(v):
        b, s, h, d = v.shape
        pos = np.arange(s, dtype=np.float32)[:, None]
        inv = np.power(10000.0, -2.0 * np.arange(d // 2, dtype=np.float32) / d)
        ang = pos * inv
        sin = np.repeat(np.sin(ang), 2, axis=-1)[None, :, None, :]
        cos = np.repeat(np.cos(ang), 2, axis=-1)[None, :, None, :]
        v2 = np.stack([-v[..., 1::2], v[..., ::2]], axis=-1).reshape(v.shape)
        return v * cos + v2 * sin

    def gp(x, W, b, mask, heads, use_rope, tril):
        bx, sx, _ = x.shape
        proj = (x @ W.T + b).reshape(bx, sx, heads, 2 * HD)
        qw, kw = proj[..., :HD], proj[..., HD:]
        if use_rope:
            qw, kw = rope_np(qw), rope_np(kw)
        logits = np.einsum('bmhd,bnhd->bhmn', qw, kw) * SCALE
        pad = mask[:, None, None, :]
        logits = logits * pad - (1.0 - pad) * INF
        if tril:
            logits = logits - np.tril(np.ones((sx, sx), np.float32), -1) * INF
        return logits

    def mcce(y_true, y_pred):
        bx, hx, sx, _ = y_pred.shape
        flat = y_true[..., 0].astype(np.int64) * sx + y_true[..., 1]
        yp = y_pred.reshape(bx, hx, sx * sx).astype(np.float64)
        total = 0.0
        for b in range(bx):
            for h in range(hx):
                f = flat[b, h]
                live = f != 0
                lv = yp[b, h][f]
                pos = np.log1p(np.sum(np.exp(-lv[live])))
                neg_terms = yp[b, h].copy()
                neg_terms[0] = -np.inf
                neg_terms[np.unique(f)] = -np.inf
                neg = np.log1p(np.sum(np.exp(neg_terms)))
                total += pos + neg
        return total

    loss = 0.0
    loss += mcce(gt_entity, gp(x, W_ent, b_ent, attention_mask, 2, True, True))
    loss += mcce(gt_head, gp(x, W_head, b_head, attention_mask, 1, False, False))
    loss += mcce(gt_tail, gp(x, W_tail, b_tail, attention_mask, 1, False, False))
    return np.array(loss, dtype=np.float32)


def _build_inputs(hidden_b, emb_rows):
    """Per-example xT8a [128,8192], xT8b [64,2048] fp8 from x_aug [1152,1024]."""
    X = np.zeros((KPAD, S), np.float32)
    X[:HID] = hidden_b.T
    X[HID:HID + LAB] = emb_rows.T
    X[HID + LAB] = 1.0
    X8 = X.astype(ml_dtypes.float8_e4m3)
    a = X8[:1024].reshape(4, 2, 128, S).transpose(2, 0, 1, 3).reshape(128, 8192)
    b = X8[1024:1152].reshape(2, 64, S).transpose(1, 0, 2).reshape(64, 2048)
    return np.ascontiguousarray(a), np.ascontiguousarray(b)


def _build_weights(W_all, b_all):
    """wtb8a [128,5120], wtb8b [64,1280] fp8 (alpha-scaled, slot-permuted)."""
    slot = _slot_map()
    W8 = np.zeros((MSLOT, KPAD), np.float32)
    live = slot >= 0
    W8[live, :HID + LAB] = ALPHA * W_all[slot[live]]
    W8[live, HID + LAB] = ALPHA * b_all[slot[live]]
    W8 = W8.astype(ml_dtypes.float8_e4m3)
    WT = np.ascontiguousarray(W8.T)                      # [1152, 640]
    a = WT[:1024].reshape(4, 2, 128, MSLOT).transpose(2, 0, 1, 3).reshape(
        128, 5120)
    b = WT[1024:1152].reshape(2, 64, MSLOT).transpose(1, 0, 2).reshape(64, 1280)
    return np.ascontiguousarray(a), np.ascontiguousarray(b)


def kernel(hidden, entity_labels, attention_mask, gt_entity, gt_head, gt_tail,
           ent_emb, W_ent, b_ent, W_head, b_head, W_tail, b_tail,
           _want_trace=False):
    hidden = np.asarray(hidden, np.float32)
    entity_labels = np.asarray(entity_labels)
    attention_mask = np.asarray(attention_mask, np.float32)
    ent_emb = np.asarray(ent_emb, np.float32)

    if not np.all(attention_mask == 1.0):
        return _reference_numpy(
            hidden, entity_labels, attention_mask, np.asarray(gt_entity),
            np.asarray(gt_head), np.asarray(gt_tail), ent_emb,
            np.asarray(W_ent, np.float32), np.asarray(b_ent, np.float32),
            np.asarray(W_head, np.float32), np.asarray(b_head, np.float32),
            np.asarray(W_tail, np.float32), np.asarray(b_tail, np.float32))

    W_all = np.concatenate(
        [np.asarray(W_ent, np.float32), np.asarray(W_head, np.float32),
         np.asarray(W_tail, np.float32)], axis=0)       # [544, 1088]
    b_all = np.concatenate(
        [np.asarray(b_ent, np.float32), np.asarray(b_head, np.float32),
         np.asarray(b_tail, np.float32)], axis=0)       # [544]

    wtb8a, wtb8b = _build_weights(W_all, b_all)
    cosT, sinT, auxm = _host_tables()

    in_maps = []
    for b in range(B):
        xa, xb = _build_inputs(hidden[b], ent_emb[entity_labels[b]])
        in_maps.append(dict(xT8a=xa, xT8b=xb, wtb8a=wtb8a, wtb8b=wtb8b,
                            cosT=cosT, sinT=sinT, aux=auxm))

    nc = _get_nc()
    res = run_bass_kernel_spmd(nc, in_maps, core_ids=list(range(NCORES)),
                               trace=_want_trace)

    # (gq, gk, tril?, sums col range, gt getter)
    heads = [
        (G_QHEAD, G_KHEAD, 0, 4, lambda b: np.asarray(gt_head)[b, 0]),
        (G_QTAIL, G_KTAIL, 4, 8, lambda b: np.asarray(gt_tail)[b, 0]),
        (G_QE0, G_KE0, 8, 11, lambda b: np.asarray(gt_entity)[b, 0]),
        (G_QE1, G_KE1, 11, 14, lambda b: np.asarray(gt_entity)[b, 1]),
    ]
    inv_a = 1.0 / ALPHA
    total = 0.0
    for b in range(B):
        out = res.results[b]
        sums_v = np.asarray(out["sums"], np.float64)       # [128, 14]
        qkv = np.asarray(out["qkout"], np.float64) * inv_a  # [8, 68, 1024]
        for (gq, gk, c0, c1, getgt) in heads:
            E = float(np.sum(sums_v[:, c0:c1]))
            total += _mcce_host(E, qkv[gq], qkv[gk], getgt(b))

    if _want_trace:
        kernel._last_results = res
    return np.array(total, dtype=np.float32)
